# revision 10
# baseline (speedup 1.0000x reference)
"""Windowed sparse attention kernel for TRN2 (8 NeuronCores).

Problem: b=1, h=16, n=16384, d=32, window w=128, nw=128 windows.
Each window of 128 queries attends to [4 memory slots | prev window | cur window]
with additive bias, tanh softcap (50), softmax.

Sharding: sequence-parallel over windows. Core c handles windows
[c*16, (c+1)*16) for all 16 heads, with a one-window k/v halo.

Device dataflow (keys on partitions, packed slot-major, bf16 matmuls):
  sim columns (4096 per head): [s1(256) | s2 | ... | s15 | s16_cur(128) |
  s0_prev(128)], where slot s's 256-col block = [cur(q_{s-1}) | prev(q_s)].
  One matmul per slot computes simT[key_s, q-cols]. DVE adds the
  (pre-arranged, mask-folded) fp32 bias in 1024-col chunks while
  evacuating PSUM. ACT applies tanh softcap + exp in wide 4096-col
  instructions; exp output is bf16. mm2: lhsT = p-slice (keys x 128
  queries, bf16), rhs = v~ (keys x 33, bf16) -> out (128 q, 33) per task;
  v~'s ones column makes col 32 the softmax denominator Z. Host combines
  the 4-slot memory attention (1.5% of keys) and normalizes.
"""

import numpy as np
import ml_dtypes

BF16 = ml_dtypes.bfloat16

B, H, N, D = 1, 16, 16384, 32
W = 128                 # window size
NW = N // W             # 128 windows
NCORES = 8
WPC = NW // NCORES      # 16 windows (tasks) per core
NSLOT = WPC + 1         # 17 k/v slots (halo)
SOFTCLAMP = 50.0
SCALE = D ** -0.5
MASK_PEN = -30000.0
SIMW = WPC * 256        # 4096 packed sim cols per head

_COMPILED = None


def _prev_col(t):
    """Column of task t's prev-role 128-wide block in the packed layout."""
    return (t - 1) * 256 + 128 if t >= 1 else 15 * 256 + 128  # t=0 -> 3968


def _cur_col(t):
    """Column of task t's cur-role 128-wide block in the packed layout."""
    return t * 256 if t <= 14 else 15 * 256  # t=15 -> 3840


def _build_bass():
    import concourse.bacc as bacc
    import concourse.tile as tile
    from concourse import mybir
    from contextlib import ExitStack

    f32 = mybir.dt.float32
    bf16 = mybir.dt.bfloat16
    nc = bacc.Bacc()

    qT = nc.declare_dram_parameter("qT", [4, 128, WPC * W], bf16, isOutput=False)
    kT = nc.declare_dram_parameter("kT", [4, 128, NSLOT * W], bf16, isOutput=False)
    vv = nc.declare_dram_parameter("vv", [H, 128, NSLOT * 33], bf16, isOutput=False)
    bT = nc.declare_dram_parameter("bT", [128, SIMW], f32, isOutput=False)
    o = nc.declare_dram_parameter("o", [H, 128, WPC * 33], f32, isOutput=True)

    with ExitStack() as ctx:
        tc = ctx.enter_context(tile.TileContext(nc))
        singles = ctx.enter_context(tc.tile_pool(name="singles", bufs=1))
        qk_pool = ctx.enter_context(tc.tile_pool(name="qk", bufs=2))
        v_pool = ctx.enter_context(tc.tile_pool(name="v", bufs=3))
        wide = ctx.enter_context(tc.tile_pool(name="wide", bufs=2))
        ow_pool = ctx.enter_context(tc.tile_pool(name="ow", bufs=2))
        t_pool = ctx.enter_context(tc.tile_pool(name="tp", bufs=1))
        sim_ps = ctx.enter_context(tc.tile_pool(name="simps", bufs=3, space="PSUM"))
        out_ps = ctx.enter_context(tc.tile_pool(name="outps", bufs=2, space="PSUM"))

        # bias in 4 independently-tracked chunk tiles, DMA'd after group-0
        # q/k so the first matmuls aren't head-of-line-blocked and each
        # DVE bias-add only waits for its own chunk.
        bias0 = singles.tile([128, 1024], f32, tag="bias0")
        bias1 = singles.tile([128, 1024], f32, tag="bias1")
        bias2 = singles.tile([128, 1024], f32, tag="bias2")
        bias3 = singles.tile([128, 1024], f32, tag="bias3")
        biasC = [bias0, bias1, bias2, bias3]

        for g in range(4):
            Qg = qk_pool.tile([128, WPC * W], bf16, tag="qg")
            nc.sync.dma_start(out=Qg[:, :], in_=qT[g])
            Kg = qk_pool.tile([128, NSLOT * W], bf16, tag="kg")
            nc.sync.dma_start(out=Kg[:, :], in_=kT[g])
            if g == 0:
                for j in range(4):
                    nc.sync.dma_start(out=biasC[j][:, :],
                                      in_=bT[:, 1024 * j:1024 * (j + 1)])
            for pi in range(2):
                # process two heads per wide tile so tanh/exp run as
                # 8192-col ACT instructions (halves per-op overhead)
                simS = wide.tile([128, 2 * SIMW], f32, tag="simS")
                pS = wide.tile([128, 2 * SIMW], bf16, tag="pS")
                Vhs = []
                for e in range(2):
                    i = 2 * pi + e
                    h = 4 * g + i
                    p0 = 32 * i
                    Vh = v_pool.tile([128, NSLOT * 33], bf16, tag="vh")
                    nc.sync.dma_start(out=Vh[:, :], in_=vv[h])
                    Vhs.append(Vh)
                    # mm1 into 4 PSUM chunks of 1024 cols (2 banks each).
                    # chunk j covers packed cols [1024j, 1024j+1024).
                    for j in range(4):
                        simP = sim_ps.tile([128, 1024], f32)
                        for s in range(4 * j + 1, 4 * j + 5):
                            # slot s block at packed col (s-1)*256:
                            off = (s - 1) * 256 - 1024 * j
                            lhsT = Kg[p0:p0 + 32, s * W:(s + 1) * W]
                            if s <= 15:
                                nc.tensor.matmul(simP[:, off:off + 256], lhsT=lhsT,
                                                 rhs=Qg[p0:p0 + 32,
                                                        (s - 1) * W:(s + 1) * W],
                                                 start=True, stop=True,
                                                 tile_position=(p0, 0))
                            else:  # s == 16: cur-role only (task 15)
                                nc.tensor.matmul(simP[:, off:off + 128], lhsT=lhsT,
                                                 rhs=Qg[p0:p0 + 32, 15 * W:16 * W],
                                                 start=True, stop=True,
                                                 tile_position=(p0, 0))
                        if j == 3:
                            # slot 0 prev-role only (task 0) at packed col 3968
                            nc.tensor.matmul(simP[:, 896:1024],
                                             lhsT=Kg[p0:p0 + 32, 0:W],
                                             rhs=Qg[p0:p0 + 32, 0:W],
                                             start=True, stop=True,
                                             tile_position=(p0, 0))
                        nc.vector.tensor_add(
                            simS[:, e * SIMW + j * 1024:e * SIMW + (j + 1) * 1024],
                            simP[:, :],
                            biasC[j][:, :],
                        )
                # softcap + exp; tS is bufs=1 (tanh->exp are consecutive
                # ACT ops, no cross-pair overlap lost). Split tanh for the
                # very first pair so ACT starts as soon as head 0 is ready.
                tS = t_pool.tile([128, 2 * SIMW], f32, tag="tS")
                if g == 0 and pi == 0:
                    nc.scalar.activation(tS[:, 0:SIMW], simS[:, 0:SIMW],
                                         mybir.ActivationFunctionType.Tanh,
                                         scale=1.0 / SOFTCLAMP)
                    nc.scalar.activation(tS[:, SIMW:], simS[:, SIMW:],
                                         mybir.ActivationFunctionType.Tanh,
                                         scale=1.0 / SOFTCLAMP)
                else:
                    nc.scalar.activation(tS[:, :], simS[:, :],
                                         mybir.ActivationFunctionType.Tanh,
                                         scale=1.0 / SOFTCLAMP)
                nc.scalar.activation(pS[:, :], tS[:, :],
                                     mybir.ActivationFunctionType.Exp,
                                     scale=SOFTCLAMP)
                # mm2: out (128 q, 33) per task, 8 tasks per PSUM bank
                for e in range(2):
                    h = 4 * g + 2 * pi + e
                    Vh = Vhs[e]
                    outW = ow_pool.tile([128, WPC * 33], f32, tag="ow")
                    for tb in range(2):
                        otP = out_ps.tile([128, 8 * 33], f32)
                        for u in range(8):
                            t = 8 * tb + u
                            pc = e * SIMW + _prev_col(t)
                            cc = e * SIMW + _cur_col(t)
                            nc.tensor.matmul(
                                otP[:, u * 33:(u + 1) * 33],
                                lhsT=pS[:, pc:pc + 128],
                                rhs=Vh[:, t * 33:(t + 1) * 33],
                                start=True, stop=False)
                            nc.tensor.matmul(
                                otP[:, u * 33:(u + 1) * 33],
                                lhsT=pS[:, cc:cc + 128],
                                rhs=Vh[:, (t + 1) * 33:(t + 2) * 33],
                                start=False, stop=True)
                        nc.vector.tensor_copy(outW[:, tb * 264:(tb + 1) * 264],
                                              otP[:, :])
                    nc.sync.dma_start(out=o[h], in_=outW[:, :])
    nc.compile()
    return nc


def _get_compiled():
    global _COMPILED
    if _COMPILED is None:
        _COMPILED = _build_bass()
    return _COMPILED


def _prep_core(c, qs, ks, vs, ab, mvec):
    """Build per-core input arrays. qs,ks,vs: (H, N, D) (qs pre-scaled)."""
    w0 = c * WPC
    qw = qs.reshape(H, NW, W, D)[:, w0:w0 + WPC]          # (H,16,128,32)
    qTc = np.ascontiguousarray(
        qw.reshape(4, 4, WPC, W, D).transpose(0, 1, 4, 2, 3).reshape(4, 128, WPC * W))

    kw = ks.reshape(H, NW, W, D)
    vw = vs.reshape(H, NW, W, D)
    khalo = np.zeros((H, NSLOT, W, D), BF16)
    vhalo = np.zeros((H, NSLOT, W, D), BF16)
    lo = w0 - 1
    src_lo = max(lo, 0)
    dst_lo = src_lo - lo
    khalo[:, dst_lo:] = kw[:, src_lo:w0 + WPC]
    vhalo[:, dst_lo:] = vw[:, src_lo:w0 + WPC]
    kTc = np.ascontiguousarray(
        khalo.reshape(4, 4, NSLOT, W, D).transpose(0, 1, 4, 2, 3).reshape(4, 128, NSLOT * W))
    vvc = np.concatenate([vhalo, np.ones((H, NSLOT, W, 1), BF16)], axis=3)
    vvc = np.ascontiguousarray(
        vvc.transpose(0, 2, 1, 3).reshape(H, 128, NSLOT * 33))

    # bias, packed layout: slot s (1..15) block at col (s-1)*256 =
    # [cur-bias(task s-1) | prev-bias(task s)]; slot 16 cur at 3840;
    # slot 0 prev at 3968. Key mask (+ structural masking of window -1)
    # folded as additive penalty; keys of block s = global window w0+s-1.
    bTc = np.zeros((128, SIMW), np.float32)                # (key, col)
    def pen(gw):
        if gw < 0:
            return np.full((W,), MASK_PEN, np.float32)
        return np.where(mvec[gw * W:(gw + 1) * W], np.float32(0),
                        np.float32(MASK_PEN))
    for s in range(1, 16):
        gw = w0 + s - 1
        base = (s - 1) * 256
        bTc[:, base:base + 128] = ab[gw, :, 128:256].T      # cur role, task s-1
        bTc[:, base + 128:base + 256] = ab[gw + 1, :, 0:128].T  # prev role, task s
        bTc[:, base:base + 256] += pen(gw)[:, None]
    bTc[:, 3840:3968] = ab[w0 + 15, :, 128:256].T + pen(w0 + 15)[:, None]
    bTc[:, 3968:4096] = ab[w0, :, 0:128].T + pen(w0 - 1)[:, None]
    return {"qT": qTc, "kT": kTc, "vv": vvc, "bT": bTc}


def _run_device(in_maps, trace=False):
    from concourse.bass_utils import run_bass_kernel_spmd
    nc = _get_compiled()
    res = run_bass_kernel_spmd(nc, in_maps, list(range(NCORES)), trace=trace)
    return res


def kernel(q, k, v, mask, attn_bias, memory_kv, _trace=False, _ret_res=False):
    q = np.asarray(q, np.float32)
    k = np.asarray(k, np.float32)
    v = np.asarray(v, np.float32)
    mask = np.asarray(mask)
    attn_bias = np.asarray(attn_bias, np.float32)
    memory_kv = np.asarray(memory_kv, np.float32)

    qs = (q[0] * np.float32(SCALE)).astype(BF16)   # (H, N, D)
    ks, vs = k[0].astype(BF16), v[0].astype(BF16)
    ab = attn_bias[0]                   # (NW, W, 2W)
    mvec = mask[0].astype(bool)         # (N,)

    in_maps = [_prep_core(c, qs, ks, vs, ab, mvec) for c in range(NCORES)]
    res = _run_device(in_maps, trace=_trace)
    outs = [r["o"] for r in res.results]             # each (H, 128, WPC*33)

    big = np.stack(outs)                              # (8, H, 128, 528)
    # (core, h, q, task, 33) -> (h, core, task, q, 33) -> (h, n, 33)
    arr = big.reshape(NCORES, H, W, WPC, 33).transpose(1, 0, 3, 2, 4)
    arr = arr.reshape(H, N, 33)
    num = arr[..., :D].astype(np.float64)             # (H, N, D)
    z = arr[..., D].astype(np.float64)                # (H, N)

    # memory-slot attention (4 keys, no bias, mask=True) on host
    mk, mv = memory_kv[0], memory_kv[1]               # (H, 4, D)
    qs64 = q[0].astype(np.float64) * SCALE
    sim_m = np.einsum('hnd,hmd->hnm', qs64, mk.astype(np.float64))
    pm = np.exp(SOFTCLAMP * np.tanh(sim_m / SOFTCLAMP))
    num = num + np.einsum('hnm,hmd->hnd', pm, mv.astype(np.float64))
    z = z + pm.sum(-1)

    out = (num / z[..., None]).astype(np.float32)[None]   # (1, H, N, D)
    if _ret_res:
        return out, res
    return out


# revision 11
# speedup vs baseline: 1.0066x; 1.0066x over previous
"""Windowed sparse attention kernel for TRN2 (8 NeuronCores).

Problem: b=1, h=16, n=16384, d=32, window w=128, nw=128 windows.
Each window of 128 queries attends to [4 memory slots | prev window | cur window]
with additive bias, tanh softcap (50), softmax.

Sharding: sequence-parallel over windows. Core c handles windows
[c*16, (c+1)*16) for all 16 heads, with a one-window k/v halo.

Device dataflow (keys on partitions, packed slot-major, bf16 matmuls):
  sim columns (4096 per head): [s1(256) | s2 | ... | s15 | s16_cur(128) |
  s0_prev(128)], where slot s's 256-col block = [cur(q_{s-1}) | prev(q_s)].
  One matmul per slot computes simT[key_s, q-cols]. DVE adds the
  (pre-arranged, mask-folded) fp32 bias in 1024-col chunks while
  evacuating PSUM. ACT applies tanh softcap + exp in wide 4096-col
  instructions; exp output is bf16. mm2: lhsT = p-slice (keys x 128
  queries, bf16), rhs = v~ (keys x 33, bf16) -> out (128 q, 33) per task;
  v~'s ones column makes col 32 the softmax denominator Z. Host combines
  the 4-slot memory attention (1.5% of keys) and normalizes.
"""

import numpy as np
import ml_dtypes

BF16 = ml_dtypes.bfloat16

B, H, N, D = 1, 16, 16384, 32
W = 128                 # window size
NW = N // W             # 128 windows
NCORES = 8
WPC = NW // NCORES      # 16 windows (tasks) per core
NSLOT = WPC + 1         # 17 k/v slots (halo)
SOFTCLAMP = 50.0
SCALE = D ** -0.5
MASK_PEN = -30000.0
SIMW = WPC * 256        # 4096 packed sim cols per head

_COMPILED = None


def _prev_col(t):
    """Column of task t's prev-role 128-wide block in the packed layout."""
    return (t - 1) * 256 + 128 if t >= 1 else 15 * 256 + 128  # t=0 -> 3968


def _cur_col(t):
    """Column of task t's cur-role 128-wide block in the packed layout."""
    return t * 256 if t <= 14 else 15 * 256  # t=15 -> 3840


def _build_bass():
    import concourse.bacc as bacc
    import concourse.tile as tile
    from concourse import mybir
    from contextlib import ExitStack

    f32 = mybir.dt.float32
    bf16 = mybir.dt.bfloat16
    nc = bacc.Bacc()

    qT = nc.declare_dram_parameter("qT", [4, 128, WPC * W], bf16, isOutput=False)
    kT = nc.declare_dram_parameter("kT", [4, 128, NSLOT * W], bf16, isOutput=False)
    vv = nc.declare_dram_parameter("vv", [H, 128, NSLOT * 33], bf16, isOutput=False)
    bT = nc.declare_dram_parameter("bT", [128, SIMW], f32, isOutput=False)
    o = nc.declare_dram_parameter("o", [H, 128, WPC * 33], f32, isOutput=True)

    with ExitStack() as ctx:
        tc = ctx.enter_context(tile.TileContext(nc))
        singles = ctx.enter_context(tc.tile_pool(name="singles", bufs=1))
        qk_pool = ctx.enter_context(tc.tile_pool(name="qk", bufs=2))
        v_pool = ctx.enter_context(tc.tile_pool(name="v", bufs=3))
        wide = ctx.enter_context(tc.tile_pool(name="wide", bufs=2))
        ow_pool = ctx.enter_context(tc.tile_pool(name="ow", bufs=2))
        t_pool = ctx.enter_context(tc.tile_pool(name="tp", bufs=1))
        sim_ps = ctx.enter_context(tc.tile_pool(name="simps", bufs=3, space="PSUM"))
        out_ps = ctx.enter_context(tc.tile_pool(name="outps", bufs=2, space="PSUM"))

        # bias in 4 independently-tracked chunk tiles, DMA'd after group-0
        # q/k so the first matmuls aren't head-of-line-blocked and each
        # DVE bias-add only waits for its own chunk.
        bias0 = singles.tile([128, 1024], f32, tag="bias0")
        bias1 = singles.tile([128, 1024], f32, tag="bias1")
        bias2 = singles.tile([128, 1024], f32, tag="bias2")
        bias3 = singles.tile([128, 1024], f32, tag="bias3")
        biasC = [bias0, bias1, bias2, bias3]

        for g in range(4):
            Qg = qk_pool.tile([128, WPC * W], bf16, tag="qg")
            nc.sync.dma_start(out=Qg[:, :], in_=qT[g])
            Kg = qk_pool.tile([128, NSLOT * W], bf16, tag="kg")
            nc.sync.dma_start(out=Kg[:, :], in_=kT[g])
            if g == 0:
                for j in range(4):
                    nc.sync.dma_start(out=biasC[j][:, :],
                                      in_=bT[:, 1024 * j:1024 * (j + 1)])
            for pi in range(2):
                # process two heads per wide tile so tanh/exp run as
                # 8192-col ACT instructions (halves per-op overhead)
                simS = wide.tile([128, 2 * SIMW], f32, tag="simS")
                pS = wide.tile([128, 2 * SIMW], bf16, tag="pS")
                Vhs = []
                for e in range(2):
                    i = 2 * pi + e
                    h = 4 * g + i
                    p0 = 32 * i
                    Vh = v_pool.tile([128, NSLOT * 33], bf16, tag="vh")
                    nc.sync.dma_start(out=Vh[:, :], in_=vv[h])
                    Vhs.append(Vh)
                    # mm1 into 4 PSUM chunks of 1024 cols (2 banks each).
                    # chunk j covers packed cols [1024j, 1024j+1024).
                    for j in range(4):
                        simP = sim_ps.tile([128, 1024], f32)
                        for s in range(4 * j + 1, 4 * j + 5):
                            # slot s block at packed col (s-1)*256:
                            off = (s - 1) * 256 - 1024 * j
                            lhsT = Kg[p0:p0 + 32, s * W:(s + 1) * W]
                            if s <= 15:
                                nc.tensor.matmul(simP[:, off:off + 256], lhsT=lhsT,
                                                 rhs=Qg[p0:p0 + 32,
                                                        (s - 1) * W:(s + 1) * W],
                                                 start=True, stop=True,
                                                 tile_position=(p0, 0))
                            else:  # s == 16: cur-role only (task 15)
                                nc.tensor.matmul(simP[:, off:off + 128], lhsT=lhsT,
                                                 rhs=Qg[p0:p0 + 32, 15 * W:16 * W],
                                                 start=True, stop=True,
                                                 tile_position=(p0, 0))
                        if j == 3:
                            # slot 0 prev-role only (task 0) at packed col 3968
                            nc.tensor.matmul(simP[:, 896:1024],
                                             lhsT=Kg[p0:p0 + 32, 0:W],
                                             rhs=Qg[p0:p0 + 32, 0:W],
                                             start=True, stop=True,
                                             tile_position=(p0, 0))
                        nc.vector.tensor_add(
                            simS[:, e * SIMW + j * 1024:e * SIMW + (j + 1) * 1024],
                            simP[:, :],
                            biasC[j][:, :],
                        )
                # softcap + exp; tS is bufs=1 (tanh->exp are consecutive
                # ACT ops, no cross-pair overlap lost). First pair: chunk
                # head 0's tanh so ACT starts as soon as the first bias-add
                # lands; split exp per head. Last pair: split exp per head
                # so head 14's mm2 overlaps head 15's exp.
                tS = t_pool.tile([128, 2 * SIMW], f32, tag="tS")
                Tanh = mybir.ActivationFunctionType.Tanh
                Exp = mybir.ActivationFunctionType.Exp
                first = g == 0 and pi == 0
                last = g == 3 and pi == 1
                if first:
                    for c in range(4):
                        nc.scalar.activation(tS[:, c * 1024:(c + 1) * 1024],
                                             simS[:, c * 1024:(c + 1) * 1024],
                                             Tanh, scale=1.0 / SOFTCLAMP)
                    nc.scalar.activation(pS[:, 0:SIMW], tS[:, 0:SIMW],
                                         Exp, scale=SOFTCLAMP)
                    nc.scalar.activation(tS[:, SIMW:], simS[:, SIMW:],
                                         Tanh, scale=1.0 / SOFTCLAMP)
                    nc.scalar.activation(pS[:, SIMW:], tS[:, SIMW:],
                                         Exp, scale=SOFTCLAMP)
                else:
                    nc.scalar.activation(tS[:, :], simS[:, :],
                                         Tanh, scale=1.0 / SOFTCLAMP)
                    if last:
                        nc.scalar.activation(pS[:, 0:SIMW], tS[:, 0:SIMW],
                                             Exp, scale=SOFTCLAMP)
                        nc.scalar.activation(pS[:, SIMW:], tS[:, SIMW:],
                                             Exp, scale=SOFTCLAMP)
                    else:
                        nc.scalar.activation(pS[:, :], tS[:, :],
                                             Exp, scale=SOFTCLAMP)
                # mm2: out (128 q, 33) per task, 8 tasks per PSUM bank
                for e in range(2):
                    h = 4 * g + 2 * pi + e
                    Vh = Vhs[e]
                    outW = ow_pool.tile([128, WPC * 33], f32, tag="ow")
                    for tb in range(2):
                        otP = out_ps.tile([128, 8 * 33], f32)
                        for u in range(8):
                            t = 8 * tb + u
                            pc = e * SIMW + _prev_col(t)
                            cc = e * SIMW + _cur_col(t)
                            nc.tensor.matmul(
                                otP[:, u * 33:(u + 1) * 33],
                                lhsT=pS[:, pc:pc + 128],
                                rhs=Vh[:, t * 33:(t + 1) * 33],
                                start=True, stop=False)
                            nc.tensor.matmul(
                                otP[:, u * 33:(u + 1) * 33],
                                lhsT=pS[:, cc:cc + 128],
                                rhs=Vh[:, (t + 1) * 33:(t + 2) * 33],
                                start=False, stop=True)
                        nc.vector.tensor_copy(outW[:, tb * 264:(tb + 1) * 264],
                                              otP[:, :])
                    nc.sync.dma_start(out=o[h], in_=outW[:, :])
    nc.compile()
    return nc


def _get_compiled():
    global _COMPILED
    if _COMPILED is None:
        _COMPILED = _build_bass()
    return _COMPILED


def _prep_core(c, qs, ks, vs, ab, mvec):
    """Build per-core input arrays. qs,ks,vs: (H, N, D) (qs pre-scaled)."""
    w0 = c * WPC
    qw = qs.reshape(H, NW, W, D)[:, w0:w0 + WPC]          # (H,16,128,32)
    qTc = np.ascontiguousarray(
        qw.reshape(4, 4, WPC, W, D).transpose(0, 1, 4, 2, 3).reshape(4, 128, WPC * W))

    kw = ks.reshape(H, NW, W, D)
    vw = vs.reshape(H, NW, W, D)
    khalo = np.zeros((H, NSLOT, W, D), BF16)
    vhalo = np.zeros((H, NSLOT, W, D), BF16)
    lo = w0 - 1
    src_lo = max(lo, 0)
    dst_lo = src_lo - lo
    khalo[:, dst_lo:] = kw[:, src_lo:w0 + WPC]
    vhalo[:, dst_lo:] = vw[:, src_lo:w0 + WPC]
    kTc = np.ascontiguousarray(
        khalo.reshape(4, 4, NSLOT, W, D).transpose(0, 1, 4, 2, 3).reshape(4, 128, NSLOT * W))
    vvc = np.concatenate([vhalo, np.ones((H, NSLOT, W, 1), BF16)], axis=3)
    vvc = np.ascontiguousarray(
        vvc.transpose(0, 2, 1, 3).reshape(H, 128, NSLOT * 33))

    # bias, packed layout: slot s (1..15) block at col (s-1)*256 =
    # [cur-bias(task s-1) | prev-bias(task s)]; slot 16 cur at 3840;
    # slot 0 prev at 3968. Key mask (+ structural masking of window -1)
    # folded as additive penalty; keys of block s = global window w0+s-1.
    bTc = np.zeros((128, SIMW), np.float32)                # (key, col)
    def pen(gw):
        if gw < 0:
            return np.full((W,), MASK_PEN, np.float32)
        return np.where(mvec[gw * W:(gw + 1) * W], np.float32(0),
                        np.float32(MASK_PEN))
    for s in range(1, 16):
        gw = w0 + s - 1
        base = (s - 1) * 256
        bTc[:, base:base + 128] = ab[gw, :, 128:256].T      # cur role, task s-1
        bTc[:, base + 128:base + 256] = ab[gw + 1, :, 0:128].T  # prev role, task s
        bTc[:, base:base + 256] += pen(gw)[:, None]
    bTc[:, 3840:3968] = ab[w0 + 15, :, 128:256].T + pen(w0 + 15)[:, None]
    bTc[:, 3968:4096] = ab[w0, :, 0:128].T + pen(w0 - 1)[:, None]
    return {"qT": qTc, "kT": kTc, "vv": vvc, "bT": bTc}


def _run_device(in_maps, trace=False):
    from concourse.bass_utils import run_bass_kernel_spmd
    nc = _get_compiled()
    res = run_bass_kernel_spmd(nc, in_maps, list(range(NCORES)), trace=trace)
    return res


def kernel(q, k, v, mask, attn_bias, memory_kv, _trace=False, _ret_res=False):
    q = np.asarray(q, np.float32)
    k = np.asarray(k, np.float32)
    v = np.asarray(v, np.float32)
    mask = np.asarray(mask)
    attn_bias = np.asarray(attn_bias, np.float32)
    memory_kv = np.asarray(memory_kv, np.float32)

    qs = (q[0] * np.float32(SCALE)).astype(BF16)   # (H, N, D)
    ks, vs = k[0].astype(BF16), v[0].astype(BF16)
    ab = attn_bias[0]                   # (NW, W, 2W)
    mvec = mask[0].astype(bool)         # (N,)

    in_maps = [_prep_core(c, qs, ks, vs, ab, mvec) for c in range(NCORES)]
    res = _run_device(in_maps, trace=_trace)
    outs = [r["o"] for r in res.results]             # each (H, 128, WPC*33)

    big = np.stack(outs)                              # (8, H, 128, 528)
    # (core, h, q, task, 33) -> (h, core, task, q, 33) -> (h, n, 33)
    arr = big.reshape(NCORES, H, W, WPC, 33).transpose(1, 0, 3, 2, 4)
    arr = arr.reshape(H, N, 33)
    num = arr[..., :D].astype(np.float64)             # (H, N, D)
    z = arr[..., D].astype(np.float64)                # (H, N)

    # memory-slot attention (4 keys, no bias, mask=True) on host
    mk, mv = memory_kv[0], memory_kv[1]               # (H, 4, D)
    qs64 = q[0].astype(np.float64) * SCALE
    sim_m = np.einsum('hnd,hmd->hnm', qs64, mk.astype(np.float64))
    pm = np.exp(SOFTCLAMP * np.tanh(sim_m / SOFTCLAMP))
    num = num + np.einsum('hnm,hmd->hnd', pm, mv.astype(np.float64))
    z = z + pm.sum(-1)

    out = (num / z[..., None]).astype(np.float32)[None]   # (1, H, N, D)
    if _ret_res:
        return out, res
    return out


# revision 12
# speedup vs baseline: 1.0135x; 1.0069x over previous
"""Windowed sparse attention kernel for TRN2 (8 NeuronCores).

Problem: b=1, h=16, n=16384, d=32, window w=128, nw=128 windows.
Each window of 128 queries attends to [4 memory slots | prev window | cur window]
with additive bias, tanh softcap (50), softmax.

Sharding: sequence-parallel over windows. Core c handles windows
[c*16, (c+1)*16) for all 16 heads, with a one-window k/v halo.

Device dataflow (keys on partitions, packed slot-major, bf16 matmuls):
  sim columns (4096 per head): [s1(256) | s2 | ... | s15 | s16_cur(128) |
  s0_prev(128)], where slot s's 256-col block = [cur(q_{s-1}) | prev(q_s)].
  One matmul per slot computes simT[key_s, q-cols]. DVE adds the
  (pre-arranged, mask-folded) fp32 bias in 1024-col chunks while
  evacuating PSUM. ACT applies tanh softcap + exp in wide 4096-col
  instructions; exp output is bf16. mm2: lhsT = p-slice (keys x 128
  queries, bf16), rhs = v~ (keys x 33, bf16) -> out (128 q, 33) per task;
  v~'s ones column makes col 32 the softmax denominator Z. Host combines
  the 4-slot memory attention (1.5% of keys) and normalizes.
"""

import numpy as np
import ml_dtypes

BF16 = ml_dtypes.bfloat16

B, H, N, D = 1, 16, 16384, 32
W = 128                 # window size
NW = N // W             # 128 windows
NCORES = 8
WPC = NW // NCORES      # 16 windows (tasks) per core
NSLOT = WPC + 1         # 17 k/v slots (halo)
SOFTCLAMP = 50.0
SCALE = D ** -0.5
MASK_PEN = -30000.0
SIMW = WPC * 256        # 4096 packed sim cols per head

_COMPILED = None


def _prev_col(t):
    """Column of task t's prev-role 128-wide block in the packed layout."""
    return (t - 1) * 256 + 128 if t >= 1 else 15 * 256 + 128  # t=0 -> 3968


def _cur_col(t):
    """Column of task t's cur-role 128-wide block in the packed layout."""
    return t * 256 if t <= 14 else 15 * 256  # t=15 -> 3840


def _build_bass():
    import concourse.bacc as bacc
    import concourse.tile as tile
    from concourse import mybir
    from contextlib import ExitStack

    f32 = mybir.dt.float32
    bf16 = mybir.dt.bfloat16
    nc = bacc.Bacc()

    qT = nc.declare_dram_parameter("qT", [4, 128, WPC * W], bf16, isOutput=False)
    kT = nc.declare_dram_parameter("kT", [4, 128, NSLOT * W], bf16, isOutput=False)
    vv = nc.declare_dram_parameter("vv", [H, 128, NSLOT * 33], bf16, isOutput=False)
    bT = nc.declare_dram_parameter("bT", [128, SIMW], bf16, isOutput=False)
    o = nc.declare_dram_parameter("o", [H, 128, WPC * 33], f32, isOutput=True)

    with ExitStack() as ctx:
        tc = ctx.enter_context(tile.TileContext(nc))
        singles = ctx.enter_context(tc.tile_pool(name="singles", bufs=1))
        qk_pool = ctx.enter_context(tc.tile_pool(name="qk", bufs=2))
        v_pool = ctx.enter_context(tc.tile_pool(name="v", bufs=3))
        wide = ctx.enter_context(tc.tile_pool(name="wide", bufs=2))
        ow_pool = ctx.enter_context(tc.tile_pool(name="ow", bufs=2))
        t_pool = ctx.enter_context(tc.tile_pool(name="tp", bufs=1))
        sim_ps = ctx.enter_context(tc.tile_pool(name="simps", bufs=3, space="PSUM"))
        out_ps = ctx.enter_context(tc.tile_pool(name="outps", bufs=2, space="PSUM"))

        # bias in 4 independently-tracked chunk tiles, DMA'd after group-0
        # q/k so the first matmuls aren't head-of-line-blocked and each
        # DVE bias-add only waits for its own chunk.
        bias0 = singles.tile([128, 1024], bf16, tag="bias0")
        bias1 = singles.tile([128, 1024], bf16, tag="bias1")
        bias2 = singles.tile([128, 1024], bf16, tag="bias2")
        bias3 = singles.tile([128, 1024], bf16, tag="bias3")
        biasC = [bias0, bias1, bias2, bias3]

        for g in range(4):
            Qg = qk_pool.tile([128, WPC * W], bf16, tag="qg")
            nc.sync.dma_start(out=Qg[:, :], in_=qT[g])
            Kg = qk_pool.tile([128, NSLOT * W], bf16, tag="kg")
            nc.sync.dma_start(out=Kg[:, :], in_=kT[g])
            if g == 0:
                for j in range(4):
                    nc.sync.dma_start(out=biasC[j][:, :],
                                      in_=bT[:, 1024 * j:1024 * (j + 1)])
            for pi in range(2):
                # process two heads per wide tile so tanh/exp run as
                # 8192-col ACT instructions (halves per-op overhead)
                simS = wide.tile([128, 2 * SIMW], f32, tag="simS")
                pS = wide.tile([128, 2 * SIMW], bf16, tag="pS")
                Vhs = []
                for e in range(2):
                    i = 2 * pi + e
                    h = 4 * g + i
                    p0 = 32 * i
                    Vh = v_pool.tile([128, NSLOT * 33], bf16, tag="vh")
                    nc.sync.dma_start(out=Vh[:, :], in_=vv[h])
                    Vhs.append(Vh)
                    # mm1 into 4 PSUM chunks of 1024 cols (2 banks each).
                    # chunk j covers packed cols [1024j, 1024j+1024).
                    for j in range(4):
                        simP = sim_ps.tile([128, 1024], f32)
                        for s in range(4 * j + 1, 4 * j + 5):
                            # slot s block at packed col (s-1)*256:
                            off = (s - 1) * 256 - 1024 * j
                            lhsT = Kg[p0:p0 + 32, s * W:(s + 1) * W]
                            if s <= 15:
                                nc.tensor.matmul(simP[:, off:off + 256], lhsT=lhsT,
                                                 rhs=Qg[p0:p0 + 32,
                                                        (s - 1) * W:(s + 1) * W],
                                                 start=True, stop=True,
                                                 tile_position=(p0, 0))
                            else:  # s == 16: cur-role only (task 15)
                                nc.tensor.matmul(simP[:, off:off + 128], lhsT=lhsT,
                                                 rhs=Qg[p0:p0 + 32, 15 * W:16 * W],
                                                 start=True, stop=True,
                                                 tile_position=(p0, 0))
                        if j == 3:
                            # slot 0 prev-role only (task 0) at packed col 3968
                            nc.tensor.matmul(simP[:, 896:1024],
                                             lhsT=Kg[p0:p0 + 32, 0:W],
                                             rhs=Qg[p0:p0 + 32, 0:W],
                                             start=True, stop=True,
                                             tile_position=(p0, 0))
                        nc.vector.tensor_add(
                            simS[:, e * SIMW + j * 1024:e * SIMW + (j + 1) * 1024],
                            simP[:, :],
                            biasC[j][:, :],
                        )
                # softcap + exp; tS is bufs=1 (tanh->exp are consecutive
                # ACT ops, no cross-pair overlap lost). First pair: chunk
                # head 0's tanh so ACT starts as soon as the first bias-add
                # lands; split exp per head. Last pair: split exp per head
                # so head 14's mm2 overlaps head 15's exp.
                tS = t_pool.tile([128, 2 * SIMW], f32, tag="tS")
                Tanh = mybir.ActivationFunctionType.Tanh
                Exp = mybir.ActivationFunctionType.Exp
                first = g == 0 and pi == 0
                last = g == 3 and pi == 1
                if first:
                    for c in range(4):
                        nc.scalar.activation(tS[:, c * 1024:(c + 1) * 1024],
                                             simS[:, c * 1024:(c + 1) * 1024],
                                             Tanh, scale=1.0 / SOFTCLAMP)
                    nc.scalar.activation(pS[:, 0:SIMW], tS[:, 0:SIMW],
                                         Exp, scale=SOFTCLAMP)
                    nc.scalar.activation(tS[:, SIMW:], simS[:, SIMW:],
                                         Tanh, scale=1.0 / SOFTCLAMP)
                    nc.scalar.activation(pS[:, SIMW:], tS[:, SIMW:],
                                         Exp, scale=SOFTCLAMP)
                else:
                    nc.scalar.activation(tS[:, :], simS[:, :],
                                         Tanh, scale=1.0 / SOFTCLAMP)
                    if last:
                        nc.scalar.activation(pS[:, 0:SIMW], tS[:, 0:SIMW],
                                             Exp, scale=SOFTCLAMP)
                        for c in (3, 0, 1, 2):
                            lo = SIMW + c * 1024
                            nc.scalar.activation(pS[:, lo:lo + 1024],
                                                 tS[:, lo:lo + 1024],
                                                 Exp, scale=SOFTCLAMP)
                    else:
                        nc.scalar.activation(pS[:, :], tS[:, :],
                                             Exp, scale=SOFTCLAMP)
                # mm2: out (128 q, 33) per task, 8 tasks per PSUM bank
                for e in range(2):
                    h = 4 * g + 2 * pi + e
                    Vh = Vhs[e]
                    outW = ow_pool.tile([128, WPC * 33], f32, tag="ow")
                    for tb in range(2):
                        otP = out_ps.tile([128, 8 * 33], f32)
                        for u in range(8):
                            t = 8 * tb + u
                            pc = e * SIMW + _prev_col(t)
                            cc = e * SIMW + _cur_col(t)
                            nc.tensor.matmul(
                                otP[:, u * 33:(u + 1) * 33],
                                lhsT=pS[:, pc:pc + 128],
                                rhs=Vh[:, t * 33:(t + 1) * 33],
                                start=True, stop=False)
                            nc.tensor.matmul(
                                otP[:, u * 33:(u + 1) * 33],
                                lhsT=pS[:, cc:cc + 128],
                                rhs=Vh[:, (t + 1) * 33:(t + 2) * 33],
                                start=False, stop=True)
                        nc.vector.tensor_copy(outW[:, tb * 264:(tb + 1) * 264],
                                              otP[:, :])
                    nc.sync.dma_start(out=o[h], in_=outW[:, :])
    nc.compile()
    return nc


def _get_compiled():
    global _COMPILED
    if _COMPILED is None:
        _COMPILED = _build_bass()
    return _COMPILED


def _prep_core(c, qs, ks, vs, ab, mvec):
    """Build per-core input arrays. qs,ks,vs: (H, N, D) (qs pre-scaled)."""
    w0 = c * WPC
    qw = qs.reshape(H, NW, W, D)[:, w0:w0 + WPC]          # (H,16,128,32)
    qTc = np.ascontiguousarray(
        qw.reshape(4, 4, WPC, W, D).transpose(0, 1, 4, 2, 3).reshape(4, 128, WPC * W))

    kw = ks.reshape(H, NW, W, D)
    vw = vs.reshape(H, NW, W, D)
    khalo = np.zeros((H, NSLOT, W, D), BF16)
    vhalo = np.zeros((H, NSLOT, W, D), BF16)
    lo = w0 - 1
    src_lo = max(lo, 0)
    dst_lo = src_lo - lo
    khalo[:, dst_lo:] = kw[:, src_lo:w0 + WPC]
    vhalo[:, dst_lo:] = vw[:, src_lo:w0 + WPC]
    kTc = np.ascontiguousarray(
        khalo.reshape(4, 4, NSLOT, W, D).transpose(0, 1, 4, 2, 3).reshape(4, 128, NSLOT * W))
    vvc = np.concatenate([vhalo, np.ones((H, NSLOT, W, 1), BF16)], axis=3)
    vvc = np.ascontiguousarray(
        vvc.transpose(0, 2, 1, 3).reshape(H, 128, NSLOT * 33))

    # bias, packed layout: slot s (1..15) block at col (s-1)*256 =
    # [cur-bias(task s-1) | prev-bias(task s)]; slot 16 cur at 3840;
    # slot 0 prev at 3968. Key mask (+ structural masking of window -1)
    # folded as additive penalty; keys of block s = global window w0+s-1.
    bTc = np.zeros((128, SIMW), np.float32)                # (key, col)
    def pen(gw):
        if gw < 0:
            return np.full((W,), MASK_PEN, np.float32)
        return np.where(mvec[gw * W:(gw + 1) * W], np.float32(0),
                        np.float32(MASK_PEN))
    for s in range(1, 16):
        gw = w0 + s - 1
        base = (s - 1) * 256
        bTc[:, base:base + 128] = ab[gw, :, 128:256].T      # cur role, task s-1
        bTc[:, base + 128:base + 256] = ab[gw + 1, :, 0:128].T  # prev role, task s
        bTc[:, base:base + 256] += pen(gw)[:, None]
    bTc[:, 3840:3968] = ab[w0 + 15, :, 128:256].T + pen(w0 + 15)[:, None]
    bTc[:, 3968:4096] = ab[w0, :, 0:128].T + pen(w0 - 1)[:, None]
    return {"qT": qTc, "kT": kTc, "vv": vvc, "bT": bTc.astype(BF16)}


def _run_device(in_maps, trace=False):
    from concourse.bass_utils import run_bass_kernel_spmd
    nc = _get_compiled()
    res = run_bass_kernel_spmd(nc, in_maps, list(range(NCORES)), trace=trace)
    return res


def kernel(q, k, v, mask, attn_bias, memory_kv, _trace=False, _ret_res=False):
    q = np.asarray(q, np.float32)
    k = np.asarray(k, np.float32)
    v = np.asarray(v, np.float32)
    mask = np.asarray(mask)
    attn_bias = np.asarray(attn_bias, np.float32)
    memory_kv = np.asarray(memory_kv, np.float32)

    qs = (q[0] * np.float32(SCALE)).astype(BF16)   # (H, N, D)
    ks, vs = k[0].astype(BF16), v[0].astype(BF16)
    ab = attn_bias[0]                   # (NW, W, 2W)
    mvec = mask[0].astype(bool)         # (N,)

    in_maps = [_prep_core(c, qs, ks, vs, ab, mvec) for c in range(NCORES)]
    res = _run_device(in_maps, trace=_trace)
    outs = [r["o"] for r in res.results]             # each (H, 128, WPC*33)

    big = np.stack(outs)                              # (8, H, 128, 528)
    # (core, h, q, task, 33) -> (h, core, task, q, 33) -> (h, n, 33)
    arr = big.reshape(NCORES, H, W, WPC, 33).transpose(1, 0, 3, 2, 4)
    arr = arr.reshape(H, N, 33)
    num = arr[..., :D].astype(np.float64)             # (H, N, D)
    z = arr[..., D].astype(np.float64)                # (H, N)

    # memory-slot attention (4 keys, no bias, mask=True) on host
    mk, mv = memory_kv[0], memory_kv[1]               # (H, 4, D)
    qs64 = q[0].astype(np.float64) * SCALE
    sim_m = np.einsum('hnd,hmd->hnm', qs64, mk.astype(np.float64))
    pm = np.exp(SOFTCLAMP * np.tanh(sim_m / SOFTCLAMP))
    num = num + np.einsum('hnm,hmd->hnd', pm, mv.astype(np.float64))
    z = z + pm.sum(-1)

    out = (num / z[..., None]).astype(np.float32)[None]   # (1, H, N, D)
    if _ret_res:
        return out, res
    return out


# revision 13
# speedup vs baseline: 1.0205x; 1.0068x over previous
"""Windowed sparse attention kernel for TRN2 (8 NeuronCores).

Problem: b=1, h=16, n=16384, d=32, window w=128, nw=128 windows.
Each window of 128 queries attends to [4 memory slots | prev window | cur window]
with additive bias, tanh softcap (50), softmax.

Sharding: sequence-parallel over windows. Core c handles windows
[c*16, (c+1)*16) for all 16 heads, with a one-window k/v halo.

Device dataflow (keys on partitions, packed slot-major, bf16 matmuls):
  sim columns (4096 per head): [s1(256) | s2 | ... | s15 | s16_cur(128) |
  s0_prev(128)], where slot s's 256-col block = [cur(q_{s-1}) | prev(q_s)].
  One matmul per slot computes simT[key_s, q-cols]. DVE adds the
  (pre-arranged, mask-folded) fp32 bias in 1024-col chunks while
  evacuating PSUM. ACT applies tanh softcap + exp in wide 4096-col
  instructions; exp output is bf16. mm2: lhsT = p-slice (keys x 128
  queries, bf16), rhs = v~ (keys x 33, bf16) -> out (128 q, 33) per task;
  v~'s ones column makes col 32 the softmax denominator Z. Host combines
  the 4-slot memory attention (1.5% of keys) and normalizes.
"""

import numpy as np
import ml_dtypes

BF16 = ml_dtypes.bfloat16

B, H, N, D = 1, 16, 16384, 32
W = 128                 # window size
NW = N // W             # 128 windows
NCORES = 8
WPC = NW // NCORES      # 16 windows (tasks) per core
NSLOT = WPC + 1         # 17 k/v slots (halo)
SOFTCLAMP = 50.0
SCALE = D ** -0.5
MASK_PEN = -30000.0
SIMW = WPC * 256        # 4096 packed sim cols per head

_COMPILED = None


def _prev_col(t):
    """Column of task t's prev-role 128-wide block in the packed layout."""
    return (t - 1) * 256 + 128 if t >= 1 else 15 * 256 + 128  # t=0 -> 3968


def _cur_col(t):
    """Column of task t's cur-role 128-wide block in the packed layout."""
    return t * 256 if t <= 14 else 15 * 256  # t=15 -> 3840


def _build_bass():
    import concourse.bacc as bacc
    import concourse.tile as tile
    from concourse import mybir
    from contextlib import ExitStack

    f32 = mybir.dt.float32
    bf16 = mybir.dt.bfloat16
    nc = bacc.Bacc()

    qT = nc.declare_dram_parameter("qT", [4, 128, WPC * W], bf16, isOutput=False)
    kT = nc.declare_dram_parameter("kT", [4, 128, NSLOT * W], bf16, isOutput=False)
    vv = nc.declare_dram_parameter("vv", [H, 128, NSLOT * 33], bf16, isOutput=False)
    bT = nc.declare_dram_parameter("bT", [128, SIMW], bf16, isOutput=False)
    o = nc.declare_dram_parameter("o", [H, 128, WPC * 33], f32, isOutput=True)

    with ExitStack() as ctx:
        tc = ctx.enter_context(tile.TileContext(nc))
        singles = ctx.enter_context(tc.tile_pool(name="singles", bufs=1))
        qk_pool = ctx.enter_context(tc.tile_pool(name="qk", bufs=2))
        v_pool = ctx.enter_context(tc.tile_pool(name="v", bufs=3))
        wide = ctx.enter_context(tc.tile_pool(name="wide", bufs=2))
        ow_pool = ctx.enter_context(tc.tile_pool(name="ow", bufs=2))
        t_pool = ctx.enter_context(tc.tile_pool(name="tp", bufs=1))
        sim_ps = ctx.enter_context(tc.tile_pool(name="simps", bufs=3, space="PSUM"))
        out_ps = ctx.enter_context(tc.tile_pool(name="outps", bufs=2, space="PSUM"))

        # bias in 4 independently-tracked chunk tiles, DMA'd after group-0
        # q/k so the first matmuls aren't head-of-line-blocked and each
        # DVE bias-add only waits for its own chunk.
        bias0 = singles.tile([128, 1024], bf16, tag="bias0")
        bias1 = singles.tile([128, 1024], bf16, tag="bias1")
        bias2 = singles.tile([128, 1024], bf16, tag="bias2")
        bias3 = singles.tile([128, 1024], bf16, tag="bias3")
        biasC = [bias0, bias1, bias2, bias3]
        # group 0's q/k split into early (slots/tasks 0-8) and late halves so
        # the first sim chunks + bias-adds start after ~1.2MB instead of 3MB
        QgA = singles.tile([128, 9 * W], bf16, tag="qgA")
        KgA = singles.tile([128, 9 * W], bf16, tag="kgA")
        QgB = singles.tile([128, 7 * W], bf16, tag="qgB")
        KgB = singles.tile([128, 8 * W], bf16, tag="kgB")

        for g in range(4):
            if g == 0:
                Qg = Kg = None
                nc.sync.dma_start(out=QgA[:, :], in_=qT[0][:, 0:9 * W])
                nc.sync.dma_start(out=KgA[:, :], in_=kT[0][:, 0:9 * W])
                nc.sync.dma_start(out=biasC[0][:, :], in_=bT[:, 0:1024])
                nc.sync.dma_start(out=biasC[1][:, :], in_=bT[:, 1024:2048])
                nc.sync.dma_start(out=QgB[:, :], in_=qT[0][:, 9 * W:16 * W])
                nc.sync.dma_start(out=KgB[:, :], in_=kT[0][:, 9 * W:17 * W])
                nc.sync.dma_start(out=biasC[2][:, :], in_=bT[:, 2048:3072])
                nc.sync.dma_start(out=biasC[3][:, :], in_=bT[:, 3072:4096])
            else:
                Qg = qk_pool.tile([128, WPC * W], bf16, tag="qg")
                nc.sync.dma_start(out=Qg[:, :], in_=qT[g])
                Kg = qk_pool.tile([128, NSLOT * W], bf16, tag="kg")
                nc.sync.dma_start(out=Kg[:, :], in_=kT[g])
            for pi in range(2):
                # process two heads per wide tile so tanh/exp run as
                # 8192-col ACT instructions (halves per-op overhead)
                simS = wide.tile([128, 2 * SIMW], f32, tag="simS")
                pS = wide.tile([128, 2 * SIMW], bf16, tag="pS")
                Vhs = []
                for e in range(2):
                    i = 2 * pi + e
                    h = 4 * g + i
                    p0 = 32 * i
                    Vh = v_pool.tile([128, NSLOT * 33], bf16, tag="vh")
                    nc.sync.dma_start(out=Vh[:, :], in_=vv[h])
                    Vhs.append(Vh)
                    # mm1 into 4 PSUM chunks of 1024 cols (2 banks each).
                    # chunk j covers packed cols [1024j, 1024j+1024).
                    def kseg(s):
                        if g != 0:
                            return Kg, s * W
                        return (KgA, s * W) if s <= 8 else (KgB, (s - 9) * W)

                    def qsegs(qlo, width):
                        # list of (tile, local_lo, w, out_off) covering
                        # q cols [qlo, qlo+width)
                        if g != 0:
                            return [(Qg, qlo, width, 0)]
                        hi = qlo + width
                        if hi <= 9 * W:
                            return [(QgA, qlo, width, 0)]
                        if qlo >= 9 * W:
                            return [(QgB, qlo - 9 * W, width, 0)]
                        w1 = 9 * W - qlo
                        return [(QgA, qlo, w1, 0), (QgB, 0, width - w1, w1)]

                    for j in range(4):
                        simP = sim_ps.tile([128, 1024], f32)
                        for s in range(4 * j + 1, 4 * j + 5):
                            # slot s block at packed col (s-1)*256:
                            off = (s - 1) * 256 - 1024 * j
                            kt, kc = kseg(s)
                            lhsT = kt[p0:p0 + 32, kc:kc + W]
                            if s <= 15:
                                segs = qsegs((s - 1) * W, 256)
                            else:  # s == 16: cur-role only (task 15)
                                segs = qsegs(15 * W, 128)
                            for qt, ql, wd, oo in segs:
                                nc.tensor.matmul(
                                    simP[:, off + oo:off + oo + wd], lhsT=lhsT,
                                    rhs=qt[p0:p0 + 32, ql:ql + wd],
                                    start=True, stop=True,
                                    tile_position=(p0, 0))
                        if j == 3:
                            # slot 0 prev-role only (task 0) at packed col 3968
                            kt, kc = kseg(0)
                            qt, ql, wd, oo = qsegs(0, 128)[0]
                            nc.tensor.matmul(simP[:, 896:1024],
                                             lhsT=kt[p0:p0 + 32, kc:kc + W],
                                             rhs=qt[p0:p0 + 32, ql:ql + 128],
                                             start=True, stop=True,
                                             tile_position=(p0, 0))
                        nc.vector.tensor_add(
                            simS[:, e * SIMW + j * 1024:e * SIMW + (j + 1) * 1024],
                            simP[:, :],
                            biasC[j][:, :],
                        )
                # softcap + exp; tS is bufs=1 (tanh->exp are consecutive
                # ACT ops, no cross-pair overlap lost). First pair: chunk
                # head 0's tanh so ACT starts as soon as the first bias-add
                # lands; split exp per head. Last pair: split exp per head
                # so head 14's mm2 overlaps head 15's exp.
                tS = t_pool.tile([128, 2 * SIMW], f32, tag="tS")
                Tanh = mybir.ActivationFunctionType.Tanh
                Exp = mybir.ActivationFunctionType.Exp
                first = g == 0 and pi == 0
                last = g == 3 and pi == 1
                if first:
                    for c in range(4):
                        nc.scalar.activation(tS[:, c * 1024:(c + 1) * 1024],
                                             simS[:, c * 1024:(c + 1) * 1024],
                                             Tanh, scale=1.0 / SOFTCLAMP)
                    nc.scalar.activation(pS[:, 0:SIMW], tS[:, 0:SIMW],
                                         Exp, scale=SOFTCLAMP)
                    nc.scalar.activation(tS[:, SIMW:], simS[:, SIMW:],
                                         Tanh, scale=1.0 / SOFTCLAMP)
                    nc.scalar.activation(pS[:, SIMW:], tS[:, SIMW:],
                                         Exp, scale=SOFTCLAMP)
                else:
                    nc.scalar.activation(tS[:, :], simS[:, :],
                                         Tanh, scale=1.0 / SOFTCLAMP)
                    if last:
                        nc.scalar.activation(pS[:, 0:SIMW], tS[:, 0:SIMW],
                                             Exp, scale=SOFTCLAMP)
                        for c in (3, 0, 1, 2):
                            lo = SIMW + c * 1024
                            nc.scalar.activation(pS[:, lo:lo + 1024],
                                                 tS[:, lo:lo + 1024],
                                                 Exp, scale=SOFTCLAMP)
                    else:
                        nc.scalar.activation(pS[:, :], tS[:, :],
                                             Exp, scale=SOFTCLAMP)
                # mm2: out (128 q, 33) per task, 8 tasks per PSUM bank
                for e in range(2):
                    h = 4 * g + 2 * pi + e
                    Vh = Vhs[e]
                    outW = ow_pool.tile([128, WPC * 33], f32, tag="ow")
                    for tb in range(2):
                        otP = out_ps.tile([128, 8 * 33], f32)
                        for u in range(8):
                            t = 8 * tb + u
                            pc = e * SIMW + _prev_col(t)
                            cc = e * SIMW + _cur_col(t)
                            nc.tensor.matmul(
                                otP[:, u * 33:(u + 1) * 33],
                                lhsT=pS[:, pc:pc + 128],
                                rhs=Vh[:, t * 33:(t + 1) * 33],
                                start=True, stop=False)
                            nc.tensor.matmul(
                                otP[:, u * 33:(u + 1) * 33],
                                lhsT=pS[:, cc:cc + 128],
                                rhs=Vh[:, (t + 1) * 33:(t + 2) * 33],
                                start=False, stop=True)
                        nc.vector.tensor_copy(outW[:, tb * 264:(tb + 1) * 264],
                                              otP[:, :])
                    nc.sync.dma_start(out=o[h], in_=outW[:, :])
    nc.compile()
    return nc


def _get_compiled():
    global _COMPILED
    if _COMPILED is None:
        _COMPILED = _build_bass()
    return _COMPILED


def _prep_core(c, qs, ks, vs, ab, mvec):
    """Build per-core input arrays. qs,ks,vs: (H, N, D) (qs pre-scaled)."""
    w0 = c * WPC
    qw = qs.reshape(H, NW, W, D)[:, w0:w0 + WPC]          # (H,16,128,32)
    qTc = np.ascontiguousarray(
        qw.reshape(4, 4, WPC, W, D).transpose(0, 1, 4, 2, 3).reshape(4, 128, WPC * W))

    kw = ks.reshape(H, NW, W, D)
    vw = vs.reshape(H, NW, W, D)
    khalo = np.zeros((H, NSLOT, W, D), BF16)
    vhalo = np.zeros((H, NSLOT, W, D), BF16)
    lo = w0 - 1
    src_lo = max(lo, 0)
    dst_lo = src_lo - lo
    khalo[:, dst_lo:] = kw[:, src_lo:w0 + WPC]
    vhalo[:, dst_lo:] = vw[:, src_lo:w0 + WPC]
    kTc = np.ascontiguousarray(
        khalo.reshape(4, 4, NSLOT, W, D).transpose(0, 1, 4, 2, 3).reshape(4, 128, NSLOT * W))
    vvc = np.concatenate([vhalo, np.ones((H, NSLOT, W, 1), BF16)], axis=3)
    vvc = np.ascontiguousarray(
        vvc.transpose(0, 2, 1, 3).reshape(H, 128, NSLOT * 33))

    # bias, packed layout: slot s (1..15) block at col (s-1)*256 =
    # [cur-bias(task s-1) | prev-bias(task s)]; slot 16 cur at 3840;
    # slot 0 prev at 3968. Key mask (+ structural masking of window -1)
    # folded as additive penalty; keys of block s = global window w0+s-1.
    bTc = np.zeros((128, SIMW), np.float32)                # (key, col)
    def pen(gw):
        if gw < 0:
            return np.full((W,), MASK_PEN, np.float32)
        return np.where(mvec[gw * W:(gw + 1) * W], np.float32(0),
                        np.float32(MASK_PEN))
    for s in range(1, 16):
        gw = w0 + s - 1
        base = (s - 1) * 256
        bTc[:, base:base + 128] = ab[gw, :, 128:256].T      # cur role, task s-1
        bTc[:, base + 128:base + 256] = ab[gw + 1, :, 0:128].T  # prev role, task s
        bTc[:, base:base + 256] += pen(gw)[:, None]
    bTc[:, 3840:3968] = ab[w0 + 15, :, 128:256].T + pen(w0 + 15)[:, None]
    bTc[:, 3968:4096] = ab[w0, :, 0:128].T + pen(w0 - 1)[:, None]
    return {"qT": qTc, "kT": kTc, "vv": vvc, "bT": bTc.astype(BF16)}


def _run_device(in_maps, trace=False):
    from concourse.bass_utils import run_bass_kernel_spmd
    nc = _get_compiled()
    res = run_bass_kernel_spmd(nc, in_maps, list(range(NCORES)), trace=trace)
    return res


def kernel(q, k, v, mask, attn_bias, memory_kv, _trace=False, _ret_res=False):
    q = np.asarray(q, np.float32)
    k = np.asarray(k, np.float32)
    v = np.asarray(v, np.float32)
    mask = np.asarray(mask)
    attn_bias = np.asarray(attn_bias, np.float32)
    memory_kv = np.asarray(memory_kv, np.float32)

    qs = (q[0] * np.float32(SCALE)).astype(BF16)   # (H, N, D)
    ks, vs = k[0].astype(BF16), v[0].astype(BF16)
    ab = attn_bias[0]                   # (NW, W, 2W)
    mvec = mask[0].astype(bool)         # (N,)

    in_maps = [_prep_core(c, qs, ks, vs, ab, mvec) for c in range(NCORES)]
    res = _run_device(in_maps, trace=_trace)
    outs = [r["o"] for r in res.results]             # each (H, 128, WPC*33)

    big = np.stack(outs)                              # (8, H, 128, 528)
    # (core, h, q, task, 33) -> (h, core, task, q, 33) -> (h, n, 33)
    arr = big.reshape(NCORES, H, W, WPC, 33).transpose(1, 0, 3, 2, 4)
    arr = arr.reshape(H, N, 33)
    num = arr[..., :D].astype(np.float64)             # (H, N, D)
    z = arr[..., D].astype(np.float64)                # (H, N)

    # memory-slot attention (4 keys, no bias, mask=True) on host
    mk, mv = memory_kv[0], memory_kv[1]               # (H, 4, D)
    qs64 = q[0].astype(np.float64) * SCALE
    sim_m = np.einsum('hnd,hmd->hnm', qs64, mk.astype(np.float64))
    pm = np.exp(SOFTCLAMP * np.tanh(sim_m / SOFTCLAMP))
    num = num + np.einsum('hnm,hmd->hnd', pm, mv.astype(np.float64))
    z = z + pm.sum(-1)

    out = (num / z[..., None]).astype(np.float32)[None]   # (1, H, N, D)
    if _ret_res:
        return out, res
    return out


# revision 16
# speedup vs baseline: 1.0374x; 1.0166x over previous
"""Windowed sparse attention kernel for TRN2 (8 NeuronCores).

Problem: b=1, h=16, n=16384, d=32, window w=128, nw=128 windows.
Each window of 128 queries attends to [4 memory slots | prev window | cur window]
with additive bias, tanh softcap (50), softmax.

Sharding: sequence-parallel over windows. Core c handles windows
[c*16, (c+1)*16) for all 16 heads, with a one-window k/v halo.

Device dataflow (keys on partitions, packed slot-major, bf16 matmuls):
  sim columns (4096 per head): [s1(256) | s2 | ... | s15 | s16_cur(128) |
  s0_prev(128)], where slot s's 256-col block = [cur(q_{s-1}) | prev(q_s)].
  One matmul per slot computes simT[key_s, q-cols]. DVE adds the
  (pre-arranged, mask-folded) fp32 bias in 1024-col chunks while
  evacuating PSUM. ACT applies tanh softcap + exp in wide 4096-col
  instructions; exp output is bf16. mm2: lhsT = p-slice (keys x 128
  queries, bf16), rhs = v~ (keys x 33, bf16) -> out (128 q, 33) per task;
  v~'s ones column makes col 32 the softmax denominator Z. Host combines
  the 4-slot memory attention (1.5% of keys) and normalizes.
"""

import numpy as np
import ml_dtypes

BF16 = ml_dtypes.bfloat16

B, H, N, D = 1, 16, 16384, 32
W = 128                 # window size
NW = N // W             # 128 windows
NCORES = 8
WPC = NW // NCORES      # 16 windows (tasks) per core
NSLOT = WPC + 1         # 17 k/v slots (halo)
SOFTCLAMP = 50.0
SCALE = D ** -0.5
MASK_PEN = -30000.0
SIMW = WPC * 256        # 4096 packed sim cols per head

_COMPILED = None


def _prev_col(t):
    """Column of task t's prev-role 128-wide block in the packed layout."""
    return (t - 1) * 256 + 128 if t >= 1 else 15 * 256 + 128  # t=0 -> 3968


def _cur_col(t):
    """Column of task t's cur-role 128-wide block in the packed layout."""
    return t * 256 if t <= 14 else 15 * 256  # t=15 -> 3840


def _build_bass():
    import concourse.bacc as bacc
    import concourse.tile as tile
    from concourse import mybir
    from contextlib import ExitStack

    f32 = mybir.dt.float32
    bf16 = mybir.dt.bfloat16
    nc = bacc.Bacc()

    qT = nc.declare_dram_parameter("qT", [4, 128, WPC * W], bf16, isOutput=False)
    kT = nc.declare_dram_parameter("kT", [4, 128, NSLOT * W], bf16, isOutput=False)
    vv = nc.declare_dram_parameter("vv", [H, 128, NSLOT * 33], bf16, isOutput=False)
    bT = nc.declare_dram_parameter("bT", [128, SIMW], bf16, isOutput=False)
    o = nc.declare_dram_parameter("o", [H, 128, WPC * 33], f32, isOutput=True)

    with ExitStack() as ctx:
        tc = ctx.enter_context(tile.TileContext(nc))
        singles = ctx.enter_context(tc.tile_pool(name="singles", bufs=1))
        qk_pool = ctx.enter_context(tc.tile_pool(name="qk", bufs=2))
        v_pool = ctx.enter_context(tc.tile_pool(name="v", bufs=4))
        wide = ctx.enter_context(tc.tile_pool(name="wide", bufs=2))
        ow_pool = ctx.enter_context(tc.tile_pool(name="ow", bufs=2))
        t_pool = ctx.enter_context(tc.tile_pool(name="tp", bufs=1))
        sim_ps = ctx.enter_context(tc.tile_pool(name="simps", bufs=3, space="PSUM"))
        out_ps = ctx.enter_context(tc.tile_pool(name="outps", bufs=2, space="PSUM"))

        # bias in 4 independently-tracked chunk tiles, DMA'd after group-0
        # q/k so the first matmuls aren't head-of-line-blocked and each
        # DVE bias-add only waits for its own chunk.
        bias0 = singles.tile([128, 1024], bf16, tag="bias0")
        bias1 = singles.tile([128, 1024], bf16, tag="bias1")
        bias2 = singles.tile([128, 1024], bf16, tag="bias2")
        bias3 = singles.tile([128, 1024], bf16, tag="bias3")
        biasC = [bias0, bias1, bias2, bias3]
        # group 0's q/k split into early (slots/tasks 0-8) and late halves so
        # the first sim chunks + bias-adds start after ~1.2MB instead of 3MB
        QgA1 = singles.tile([128, 5 * W], bf16, tag="qgA1")
        KgA1 = singles.tile([128, 5 * W], bf16, tag="kgA1")
        QgA2 = singles.tile([128, 4 * W], bf16, tag="qgA2")
        KgA2 = singles.tile([128, 4 * W], bf16, tag="kgA2")
        QgB = singles.tile([128, 7 * W], bf16, tag="qgB")
        KgB = singles.tile([128, 8 * W], bf16, tag="kgB")
        # (tile, global_lo, global_hi) maps for group 0 split loads
        KSEGS0 = [(KgA1, 0, 5 * W), (KgA2, 5 * W, 9 * W), (KgB, 9 * W, 17 * W)]
        QSEGS0 = [(QgA1, 0, 5 * W), (QgA2, 5 * W, 9 * W), (QgB, 9 * W, 16 * W)]

        def emit_mm2(pS, Vhs, h0):
            for e in range(2):
                h = h0 + e
                Vh = Vhs[e]
                outW = ow_pool.tile([128, WPC * 33], f32, tag="ow")
                for tb in range(2):
                    otP = out_ps.tile([128, 8 * 33], f32, tag="ot")
                    for u in range(8):
                        t = 8 * tb + u
                        pc = e * SIMW + _prev_col(t)
                        cc = e * SIMW + _cur_col(t)
                        nc.tensor.matmul(
                            otP[:, u * 33:(u + 1) * 33],
                            lhsT=pS[:, pc:pc + 128],
                            rhs=Vh[:, t * 33:(t + 1) * 33],
                            start=True, stop=False)
                        nc.tensor.matmul(
                            otP[:, u * 33:(u + 1) * 33],
                            lhsT=pS[:, cc:cc + 128],
                            rhs=Vh[:, (t + 1) * 33:(t + 2) * 33],
                            start=False, stop=True)
                    nc.vector.tensor_copy(outW[:, tb * 264:(tb + 1) * 264],
                                          otP[:, :])
                nc.sync.dma_start(out=o[h], in_=outW[:, :])

        pending = None
        for g in range(4):
            if g == 0:
                Qg = Kg = None
                nc.sync.dma_start(out=QgA1[:, :], in_=qT[0][:, 0:5 * W])
                nc.sync.dma_start(out=KgA1[:, :], in_=kT[0][:, 0:5 * W])
                nc.sync.dma_start(out=biasC[0][:, :], in_=bT[:, 0:1024])
                nc.sync.dma_start(out=QgA2[:, :], in_=qT[0][:, 5 * W:9 * W])
                nc.sync.dma_start(out=KgA2[:, :], in_=kT[0][:, 5 * W:9 * W])
                nc.sync.dma_start(out=biasC[1][:, :], in_=bT[:, 1024:2048])
                nc.sync.dma_start(out=QgB[:, :], in_=qT[0][:, 9 * W:16 * W])
                nc.sync.dma_start(out=KgB[:, :], in_=kT[0][:, 9 * W:17 * W])
                nc.sync.dma_start(out=biasC[2][:, :], in_=bT[:, 2048:3072])
                nc.sync.dma_start(out=biasC[3][:, :], in_=bT[:, 3072:4096])
            else:
                Qg = qk_pool.tile([128, WPC * W], bf16, tag="qg")
                nc.sync.dma_start(out=Qg[:, :], in_=qT[g])
                Kg = qk_pool.tile([128, NSLOT * W], bf16, tag="kg")
                nc.sync.dma_start(out=Kg[:, :], in_=kT[g])
            for pi in range(2):
                # process two heads per wide tile so tanh/exp run as
                # 8192-col ACT instructions (halves per-op overhead)
                simS = wide.tile([128, 2 * SIMW], f32, tag="simS")
                pS = wide.tile([128, 2 * SIMW], bf16, tag="pS")
                Vhs = []
                for e in range(2):
                    i = 2 * pi + e
                    h = 4 * g + i
                    p0 = 32 * i
                    Vh = v_pool.tile([128, NSLOT * 33], bf16, tag="vh")
                    nc.sync.dma_start(out=Vh[:, :], in_=vv[h])
                    Vhs.append(Vh)
                    # mm1 into 4 PSUM chunks of 1024 cols (2 banks each).
                    # chunk j covers packed cols [1024j, 1024j+1024).
                    def kseg(s):
                        if g != 0:
                            return Kg, s * W
                        for t, lo, hi in KSEGS0:
                            if s * W >= lo and (s + 1) * W <= hi:
                                return t, s * W - lo
                        raise AssertionError(s)

                    def qsegs(qlo, width):
                        # list of (tile, local_lo, w, out_off) covering
                        # q cols [qlo, qlo+width)
                        if g != 0:
                            return [(Qg, qlo, width, 0)]
                        out = []
                        pos = qlo
                        while pos < qlo + width:
                            for t, lo, hi in QSEGS0:
                                if lo <= pos < hi:
                                    w = min(qlo + width, hi) - pos
                                    out.append((t, pos - lo, w, pos - qlo))
                                    pos += w
                                    break
                            else:
                                raise AssertionError(pos)
                        return out

                    for j in range(4):
                        simP = sim_ps.tile([128, 1024], f32)
                        for s in range(4 * j + 1, 4 * j + 5):
                            # slot s block at packed col (s-1)*256:
                            off = (s - 1) * 256 - 1024 * j
                            kt, kc = kseg(s)
                            lhsT = kt[p0:p0 + 32, kc:kc + W]
                            if s <= 15:
                                segs = qsegs((s - 1) * W, 256)
                            else:  # s == 16: cur-role only (task 15)
                                segs = qsegs(15 * W, 128)
                            for qt, ql, wd, oo in segs:
                                nc.tensor.matmul(
                                    simP[:, off + oo:off + oo + wd], lhsT=lhsT,
                                    rhs=qt[p0:p0 + 32, ql:ql + wd],
                                    start=True, stop=True,
                                    tile_position=(p0, 0))
                        if j == 3:
                            # slot 0 prev-role only (task 0) at packed col 3968
                            kt, kc = kseg(0)
                            qt, ql, wd, oo = qsegs(0, 128)[0]
                            nc.tensor.matmul(simP[:, 896:1024],
                                             lhsT=kt[p0:p0 + 32, kc:kc + W],
                                             rhs=qt[p0:p0 + 32, ql:ql + 128],
                                             start=True, stop=True,
                                             tile_position=(p0, 0))
                        nc.vector.tensor_add(
                            simS[:, e * SIMW + j * 1024:e * SIMW + (j + 1) * 1024],
                            simP[:, :],
                            biasC[j][:, :],
                        )
                # softcap + exp; tS is bufs=1 (tanh->exp are consecutive
                # ACT ops, no cross-pair overlap lost). First pair: chunk
                # head 0's tanh so ACT starts as soon as the first bias-add
                # lands; split exp per head. Last pair: split exp per head
                # so head 14's mm2 overlaps head 15's exp.
                tS = t_pool.tile([128, 2 * SIMW], f32, tag="tS")
                Tanh = mybir.ActivationFunctionType.Tanh
                Exp = mybir.ActivationFunctionType.Exp
                first = g == 0 and pi == 0
                last = g == 3 and pi == 1
                if first:
                    for c in range(4):
                        nc.scalar.activation(tS[:, c * 1024:(c + 1) * 1024],
                                             simS[:, c * 1024:(c + 1) * 1024],
                                             Tanh, scale=1.0 / SOFTCLAMP)
                    nc.scalar.activation(pS[:, 0:SIMW], tS[:, 0:SIMW],
                                         Exp, scale=SOFTCLAMP)
                    nc.scalar.activation(tS[:, SIMW:], simS[:, SIMW:],
                                         Tanh, scale=1.0 / SOFTCLAMP)
                    nc.scalar.activation(pS[:, SIMW:], tS[:, SIMW:],
                                         Exp, scale=SOFTCLAMP)
                else:
                    nc.scalar.activation(tS[:, :], simS[:, :],
                                         Tanh, scale=1.0 / SOFTCLAMP)
                    if last:
                        nc.scalar.activation(pS[:, 0:SIMW], tS[:, 0:SIMW],
                                             Exp, scale=SOFTCLAMP)
                        for c in (3, 0, 1, 2):
                            lo = SIMW + c * 1024
                            nc.scalar.activation(pS[:, lo:lo + 1024],
                                                 tS[:, lo:lo + 1024],
                                                 Exp, scale=SOFTCLAMP)
                    else:
                        nc.scalar.activation(pS[:, :], tS[:, :],
                                             Exp, scale=SOFTCLAMP)
                # mm2 for the PREVIOUS pair (software-pipelined one pair
                # behind so the ramp's DVE bias-adds run copy-free)
                if pending is not None:
                    emit_mm2(*pending)
                pending = (pS, Vhs, 4 * g + 2 * pi)
        emit_mm2(*pending)
    nc.compile()
    return nc


def _get_compiled():
    global _COMPILED
    if _COMPILED is None:
        _COMPILED = _build_bass()
    return _COMPILED


def _prep_core(c, qs, ks, vs, ab, mvec):
    """Build per-core input arrays. qs,ks,vs: (H, N, D) (qs pre-scaled)."""
    w0 = c * WPC
    qw = qs.reshape(H, NW, W, D)[:, w0:w0 + WPC]          # (H,16,128,32)
    qTc = np.ascontiguousarray(
        qw.reshape(4, 4, WPC, W, D).transpose(0, 1, 4, 2, 3).reshape(4, 128, WPC * W))

    kw = ks.reshape(H, NW, W, D)
    vw = vs.reshape(H, NW, W, D)
    khalo = np.zeros((H, NSLOT, W, D), BF16)
    vhalo = np.zeros((H, NSLOT, W, D), BF16)
    lo = w0 - 1
    src_lo = max(lo, 0)
    dst_lo = src_lo - lo
    khalo[:, dst_lo:] = kw[:, src_lo:w0 + WPC]
    vhalo[:, dst_lo:] = vw[:, src_lo:w0 + WPC]
    kTc = np.ascontiguousarray(
        khalo.reshape(4, 4, NSLOT, W, D).transpose(0, 1, 4, 2, 3).reshape(4, 128, NSLOT * W))
    vvc = np.concatenate([vhalo, np.ones((H, NSLOT, W, 1), BF16)], axis=3)
    vvc = np.ascontiguousarray(
        vvc.transpose(0, 2, 1, 3).reshape(H, 128, NSLOT * 33))

    # bias, packed layout: slot s (1..15) block at col (s-1)*256 =
    # [cur-bias(task s-1) | prev-bias(task s)]; slot 16 cur at 3840;
    # slot 0 prev at 3968. Key mask (+ structural masking of window -1)
    # folded as additive penalty; keys of block s = global window w0+s-1.
    bTc = np.zeros((128, SIMW), np.float32)                # (key, col)
    def pen(gw):
        if gw < 0:
            return np.full((W,), MASK_PEN, np.float32)
        return np.where(mvec[gw * W:(gw + 1) * W], np.float32(0),
                        np.float32(MASK_PEN))
    for s in range(1, 16):
        gw = w0 + s - 1
        base = (s - 1) * 256
        bTc[:, base:base + 128] = ab[gw, :, 128:256].T      # cur role, task s-1
        bTc[:, base + 128:base + 256] = ab[gw + 1, :, 0:128].T  # prev role, task s
        bTc[:, base:base + 256] += pen(gw)[:, None]
    bTc[:, 3840:3968] = ab[w0 + 15, :, 128:256].T + pen(w0 + 15)[:, None]
    bTc[:, 3968:4096] = ab[w0, :, 0:128].T + pen(w0 - 1)[:, None]
    return {"qT": qTc, "kT": kTc, "vv": vvc, "bT": bTc.astype(BF16)}


def _run_device(in_maps, trace=False):
    from concourse.bass_utils import run_bass_kernel_spmd
    nc = _get_compiled()
    res = run_bass_kernel_spmd(nc, in_maps, list(range(NCORES)), trace=trace)
    return res


def kernel(q, k, v, mask, attn_bias, memory_kv, _trace=False, _ret_res=False):
    q = np.asarray(q, np.float32)
    k = np.asarray(k, np.float32)
    v = np.asarray(v, np.float32)
    mask = np.asarray(mask)
    attn_bias = np.asarray(attn_bias, np.float32)
    memory_kv = np.asarray(memory_kv, np.float32)

    qs = (q[0] * np.float32(SCALE)).astype(BF16)   # (H, N, D)
    ks, vs = k[0].astype(BF16), v[0].astype(BF16)
    ab = attn_bias[0]                   # (NW, W, 2W)
    mvec = mask[0].astype(bool)         # (N,)

    in_maps = [_prep_core(c, qs, ks, vs, ab, mvec) for c in range(NCORES)]
    res = _run_device(in_maps, trace=_trace)
    outs = [r["o"] for r in res.results]             # each (H, 128, WPC*33)

    big = np.stack(outs)                              # (8, H, 128, 528)
    # (core, h, q, task, 33) -> (h, core, task, q, 33) -> (h, n, 33)
    arr = big.reshape(NCORES, H, W, WPC, 33).transpose(1, 0, 3, 2, 4)
    arr = arr.reshape(H, N, 33)
    num = arr[..., :D].astype(np.float64)             # (H, N, D)
    z = arr[..., D].astype(np.float64)                # (H, N)

    # memory-slot attention (4 keys, no bias, mask=True) on host
    mk, mv = memory_kv[0], memory_kv[1]               # (H, 4, D)
    qs64 = q[0].astype(np.float64) * SCALE
    sim_m = np.einsum('hnd,hmd->hnm', qs64, mk.astype(np.float64))
    pm = np.exp(SOFTCLAMP * np.tanh(sim_m / SOFTCLAMP))
    num = num + np.einsum('hnm,hmd->hnd', pm, mv.astype(np.float64))
    z = z + pm.sum(-1)

    out = (num / z[..., None]).astype(np.float32)[None]   # (1, H, N, D)
    if _ret_res:
        return out, res
    return out
